# revision 14
# baseline (speedup 1.0000x reference)
"""Trainium2 Bass kernel for nn_AvgPoolVectorsPerWSI (segment-mean over groups).

Math: x [N=2048, M=512, 7, 7], idx [N] in [0,64)
  out[g, m] = mean over {n: idx[n]==g} and spatial of x[n, m, :, :]  -> [64, 512, 1, 1]

Strategy (no collectives needed):
  - Shard over M: core k handles an m-slice of 64 channels. Each core reads
    its x slice [2048, 64, 49] (25.7 MB) once -> memory-bound (~63 us/core
    at the observed ~408 GB/s DMA stream rate).
  - Per 128-row n-tile: VectorE reduces spatial j (fp32 exact, hidden under
    the DMA stream), then one small fp32 matmul with the scale-weighted
    one-hot segment matrix accumulates the output into PSUM:
      psum[g, m] += w[n, g]^T @ xs[n, m],  w[n, g] = (idx[n]==g)/(count_g*49)
  - Epilogue: copy PSUM -> SBUF, DMA out [64, 64]. Host concatenates the 8
    results along m.
  - The last n-tile is processed as two m-halves (separate DMAs/reduces/
    matmuls) to shorten the post-stream tail.

Raw Block implementation (not Tile): the walrus matmul/DMA lowerings only
accept ONE attached sync-wait per instruction; standalone wait_ge
instructions sidestep that. One semaphore per x-tile DMA: a cumulative
count over a shared sem can be satisfied by LATER tiles' completions while
tile t still has a lagging SDMA engine (8-partition stripe) in flight.
"""

from contextlib import ExitStack

import numpy as np

import concourse.bass as bass
import concourse.mybir as mybir
from concourse.bass_utils import run_bass_kernel_spmd

N = 2048          # samples
M = 512           # channels
HW = 49           # spatial (7*7)
G = 64            # groups
CORES = 8
ML = M // CORES   # 64 channels per core
F = ML * HW       # 3136 floats per (n, core)
FH = F // 2       # half-tile free size (m-split)
MH = ML // 2      # half-tile channels
P = 128           # partitions per tile
NT = N // P       # 16 n-tiles
LAST = NT - 1
BUFS = 6          # x-tile double-buffer depth
XBUFS = 4         # reduced-tile (xs) buffer depth

F32 = mybir.dt.float32


def _build():
    nc = bass.Bass(trn_type="TRN2", target_bir_lowering=False)
    x_ext = nc.declare_dram_parameter("x", [N, F], F32, isOutput=False)
    w_ext = nc.declare_dram_parameter("w", [P, NT * G], F32, isOutput=False)
    out_ext = nc.declare_dram_parameter("out", [G, ML], F32, isOutput=True)

    x_t = x_ext.ap().rearrange("(t p) f -> t p f", p=P)  # [16, 128, 3136]

    with ExitStack() as ctx:
        x_buf = ctx.enter_context(nc.sbuf_tensor([P, BUFS * F], F32))
        xs_buf = ctx.enter_context(nc.sbuf_tensor([P, XBUFS * ML], F32))
        w_sb = ctx.enter_context(nc.sbuf_tensor([P, NT * G], F32))
        out_sb = ctx.enter_context(nc.sbuf_tensor([G, ML], F32))
        psum = ctx.enter_context(nc.psum_tensor([G, ML], F32))
        # one sem per x DMA; the last tile has two (one per m-half)
        dma_x = [
            ctx.enter_context(nc.semaphore(name=f"dma_x{t}")) for t in range(NT)
        ]
        dma_xl = ctx.enter_context(nc.semaphore(name="dma_xl"))  # last half-2
        dma_w = ctx.enter_context(nc.semaphore())   # +16 when w resident
        dma_o = ctx.enter_context(nc.semaphore())   # +16 when out written
        red_sem = ctx.enter_context(nc.semaphore())  # +1 per reduce issued
        pe_sem = ctx.enter_context(nc.semaphore())   # +1 per matmul
        fin_sem = ctx.enter_context(nc.semaphore())  # +1 when out_sb ready
        block = ctx.enter_context(nc.Block())

        # ---- DMA program (SP / HWDGE, FIFO) ----
        @block.sync
        def _(sync):
            def xdma(t):
                if t >= BUFS:
                    # slot reuse: wait until tile t-BUFS fully reduced
                    # (reduce count: one per tile for 0..NT-2)
                    sync.wait_ge(red_sem, t - BUFS + 1)
                slot = t % BUFS
                if t == LAST:
                    # split last tile into two m-halves for a shorter tail
                    sync.dma_start(
                        out=x_buf[:, slot * F:slot * F + FH],
                        in_=x_t[t][:, 0:FH],
                    ).then_inc(dma_x[t], 16)
                    sync.dma_start(
                        out=x_buf[:, slot * F + FH:(slot + 1) * F],
                        in_=x_t[t][:, FH:F],
                    ).then_inc(dma_xl, 16)
                else:
                    sync.dma_start(
                        out=x_buf[:, slot * F:(slot + 1) * F], in_=x_t[t]
                    ).then_inc(dma_x[t], 16)

            xdma(0)
            xdma(1)
            sync.dma_start(out=w_sb[:, :], in_=w_ext.ap()).then_inc(dma_w, 16)
            for t in range(2, NT):
                xdma(t)
            sync.wait_ge(fin_sem, 1)
            sync.dma_start(out=out_ext.ap(), in_=out_sb[:, :]).then_inc(dma_o, 16)
            sync.wait_ge(dma_o, 16)

        # ---- spatial j-reduction (VectorE, fp32 exact) ----
        @block.vector
        def _(vector):
            def reduce(t, xslot, fr, mr):
                slot = t % BUFS
                vector.tensor_reduce(
                    out=xs_buf[:, xslot * ML + mr[0]:xslot * ML + mr[1]],
                    in_=x_buf[:, slot * F + fr[0]:slot * F + fr[1]].rearrange(
                        "p (m j) -> p m j", j=HW
                    ),
                    axis=mybir.AxisListType.X,
                    op=mybir.AluOpType.add,
                ).then_inc(red_sem, 1)

            for t in range(NT):
                vector.wait_ge(dma_x[t], 16)
                if t >= XBUFS:
                    # xs slot reuse: wait until tile t-XBUFS consumed by PE
                    vector.wait_ge(pe_sem, t - XBUFS + 1)
                xslot = t % XBUFS
                if t == LAST:
                    reduce(t, xslot, (0, FH), (0, MH))
                    vector.wait_ge(dma_xl, 16)
                    reduce(t, xslot, (FH, F), (MH, ML))
                else:
                    reduce(t, xslot, (0, F), (0, ML))
            # epilogue: copy the finished PSUM accumulator out
            vector.wait_ge(pe_sem, NT + 1)
            vector.tensor_copy(out_sb[:, :], psum[:, :]).then_inc(fin_sem, 1)

        # ---- segment-sum accumulation (TensorE, fp32) ----
        @block.tensor
        def _(tensor):
            tensor.wait_ge(dma_w, 16)
            for t in range(NT):
                xslot = t % XBUFS
                if t == LAST:
                    tensor.wait_ge(red_sem, t + 1)
                    tensor.matmul(
                        out=psum[:, 0:MH],
                        lhsT=w_sb[:, t * G:(t + 1) * G],
                        rhs=xs_buf[:, xslot * ML:xslot * ML + MH],
                        start=False,
                        stop=True,
                    ).then_inc(pe_sem, 1)
                    tensor.wait_ge(red_sem, t + 2)
                    tensor.matmul(
                        out=psum[:, MH:ML],
                        lhsT=w_sb[:, t * G:(t + 1) * G],
                        rhs=xs_buf[:, xslot * ML + MH:(xslot + 1) * ML],
                        start=False,
                        stop=True,
                    ).then_inc(pe_sem, 1)
                else:
                    tensor.wait_ge(red_sem, t + 1)
                    tensor.matmul(
                        out=psum[:, :],
                        lhsT=w_sb[:, t * G:(t + 1) * G],
                        rhs=xs_buf[:, xslot * ML:(xslot + 1) * ML],
                        start=(t == 0),
                        stop=False,
                    ).then_inc(pe_sem, 1)

    return nc


def _prepare(x, idx):
    x = np.asarray(x)
    if x.dtype != np.float32:
        x = x.astype(np.float32)
    idx = np.asarray(idx).astype(np.int64)
    counts = np.bincount(idx, minlength=G).astype(np.float64)
    scale = np.where(counts > 0, 1.0 / (counts * HW), 0.0).astype(np.float32)
    # scale-weighted one-hot; fp32 matmul keeps this exact
    w_full = np.zeros((N, G), np.float32)
    w_full[np.arange(N), idx] = scale[idx]
    # device layout: w[p, t*G + g] = w_full[t*128 + p, g]
    w_host = np.ascontiguousarray(
        w_full.reshape(NT, P, G).transpose(1, 0, 2).reshape(P, NT * G)
    )
    xr = x.reshape(N, M, HW)
    in_maps = []
    for k in range(CORES):
        shard = np.ascontiguousarray(xr[:, k * ML:(k + 1) * ML, :]).reshape(N, F)
        in_maps.append({"x": shard, "w": w_host})
    return in_maps


def run(x, tensor_list_assignmentindices, trace=False):
    in_maps = _prepare(x, tensor_list_assignmentindices)
    nc = _build()
    res = run_bass_kernel_spmd(nc, in_maps, core_ids=list(range(CORES)), trace=trace)
    outs = [np.asarray(r["out"]) for r in res.results]
    out = np.concatenate(outs, axis=1)  # [G, M]
    return out.reshape(G, M, 1, 1).astype(np.float32), res.exec_time_ns


def kernel(**inputs):
    out, _ = run(inputs["x"], inputs["tensor_list_assignmentindices"], trace=False)
    return out
